# revision 1
# baseline (speedup 1.0000x reference)
"""2-layer LSTM decoder Bass/Tile kernel for TRN2.

Per-core: B_local=128 batch rows (one SBUF partition width), H=512, 64 steps.
Data-parallel over 8 cores; weights replicated.

Layout: batch on partitions, gates on free dim ("Layout A").
  gates[b, 4H] = h @ W_hh.T + in_aug @ W_ih_aug.T
  matmul(out=gates_psum, lhsT=h.T chunks (from PE transposes), rhs=W.T chunks)
Gate column order is permuted to [i, f, o, g] so sigmoid covers a contiguous
[0:1536] span and tanh [1536:2048].
All matmuls run in float32r (TF32-like, 1 cycle/row at N>=512).
"""

import numpy as np
from contextlib import ExitStack

import concourse.bass as bass
import concourse.bacc as bacc
import concourse.mybir as mybir
import concourse.tile as tile
from concourse import masks

F32 = mybir.dt.float32
F32R = mybir.dt.float32r
AF = mybir.ActivationFunctionType
OP = mybir.AluOpType

P = 128          # batch rows per core / partition width
H = 512          # hidden
G = 2048         # 4*H gates
NT = 4           # n-tiles of 512 across gates
KC = 4           # K chunks of 128 across H
EXO = 8
ZD = 16
# gate reorder: source gate block g (pytorch i,f,g,o) -> dest block in [i,f,o,g]
NEWPOS = [0, 1, 3, 2]
PER_TILE = True  # per-tile matmul groups + per-tile ACT vs bulk emission


def r(ap):
    return ap.bitcast(F32R)


def f(ap):
    return ap.bitcast(F32)


def build_kernel(nc: bass.Bass, steps: int, repeat: int = 1):
    """Emit the full kernel (inputs/outputs declared on nc)."""
    S = steps
    # ---- DRAM params ----
    di = lambda name, shape: nc.dram_tensor(name, shape, F32, kind="ExternalInput").ap()
    y0 = di("y0", [P, 1])
    xf = di("x_future", [P, S * EXO])         # host passes reshaped (P, S*EXO)
    h00 = di("h00", [P, H])
    h01 = di("h01", [P, H])
    c00 = di("c00", [P, H])
    c01 = di("c01", [P, H])
    z = di("z", [P, ZD])
    Wih0 = di("W_ih0", [G, 9])
    Whh0 = di("W_hh0", [G, H])
    bih0 = di("b_ih0", [1, G])
    bhh0 = di("b_hh0", [1, G])
    Wih1 = di("W_ih1", [G, H])
    Whh1 = di("W_hh1", [G, H])
    bih1 = di("b_ih1", [1, G])
    bhh1 = di("b_hh1", [1, G])
    Wp = di("W_proj", [1, H])
    bp = di("b_proj", [1, 1])
    Wz = di("W_z", [9, ZD])
    bz = di("b_z", [1, 9])
    out_d = nc.dram_tensor("out", [S, P], F32, kind="ExternalOutput").ap()

    with tile.TileContext(nc) as tc, ExitStack() as ctx:
        emit(ctx, tc, nc, S, locals(), repeat)
    return nc


def emit(ctx, tc, nc, S, t_in, repeat=1):
    xf, y0, z = t_in["xf"], t_in["y0"], t_in["z"]
    h00, h01, c00, c01 = t_in["h00"], t_in["h01"], t_in["c00"], t_in["c01"]
    Wih0, Whh0, Wih1, Whh1 = t_in["Wih0"], t_in["Whh0"], t_in["Wih1"], t_in["Whh1"]
    bih0, bhh0, bih1, bhh1 = t_in["bih0"], t_in["bhh0"], t_in["bih1"], t_in["bhh1"]
    Wp, bp, Wz, bz = t_in["Wp"], t_in["bp"], t_in["Wz"], t_in["bz"]
    out_d = t_in["out_d"]

    # ---- pools ----
    const = ctx.enter_context(tc.tile_pool(name="const", bufs=1))
    state = ctx.enter_context(tc.tile_pool(name="state", bufs=2))
    gact = ctx.enter_context(tc.tile_pool(name="gact", bufs=2))
    yo = ctx.enter_context(tc.tile_pool(name="yo", bufs=3))
    ldtmp = ctx.enter_context(tc.tile_pool(name="ldtmp", bufs=2))
    gsig = ctx.enter_context(tc.tile_pool(name="gsig", bufs=2, space="PSUM"))
    gtan = ctx.enter_context(tc.tile_pool(name="gtan", bufs=1, space="PSUM"))
    tpp = ctx.enter_context(tc.tile_pool(name="tpp", bufs=1, space="PSUM"))

    # ---- persistent SBUF ----
    WhhT0 = const.tile([P, KC * G], F32R, name="WhhT0")
    WihT1 = const.tile([P, KC * G], F32R, name="WihT1")
    WhhT1 = const.tile([P, KC * G], F32R, name="WhhT1")
    # packA layout (free offsets):
    #   [0:S*128]                 p0-9   : inT_all (exo rows 0-7, ones row 8, y row 9)
    #   [S*128 : S*128+2048]      p0-9   : Wih0a10 (rhs for K=10 input matmul)
    #   [0:128]                   p32    : ones_row (lhsT for bias matmul)
    #   [128:2176]                p32    : b1row (rhs for bias matmul)
    #   [S*128+2048 : +4]         p0-127 : WpT  (proj lhsT chunks)
    #   then [1,128] rows: bp_row, ybias_row
    oWA = S * 128
    oWpT = oWA + G
    packA = const.tile([P, oWpT + 32], F32R, name="packA")
    # packA is read-only during the step loop (y rows live in rotating
    # tiles); exo rows at partitions 32-39, ones row at partition 40 so the
    # K=9 exo matmul has no y dependency; wcol/ones0 tucked under p0.
    inT_x = packA[32:41, 0 : S * 128]
    Wih0a9 = packA[32:41, oWA : oWA + G]
    wcol_row = packA[0:1, 0:G]
    ones0 = packA[0:1, G : G + 128]
    ones_row = packA[64:65, 0:128]
    b1row = packA[64:65, 128 : 128 + G]
    WpT = packA[:, oWpT : oWpT + 4]
    bp_row = const.tile([1, P], F32, name="bp_row")
    ybias_row = const.tile([1, P], F32, name="ybias_row")

    ident_f = const.tile([P, P], F32, name="ident_f")
    masks.make_identity(nc, ident_f[:])
    ident = const.tile([P, P], F32R, name="ident")
    nc.scalar.copy(ident[:], ident_f[:])
    idr = ident

    zb_sb = const.tile([P, 16], F32R, name="zb_sb")
    WzT = const.tile([16, 16], F32R, name="WzT")
    bz_sb = const.tile([1, 16], F32R, name="bz_sb")

    dma = nc.sync.dma_start
    ones_f = const.tile([1, P], F32, name="ones_f")
    nc.vector.memset(ones_f[:], 1.0)
    nc.scalar.copy(ones0, ones_f[:])

    # =========================== INIT ===========================
    # -- z bias: zb = z @ Wz.T + bz  ([P, 9]) --
    z_sb = ldtmp.tile([P, ZD], F32R, name="z_sb", tag="ldtmp")
    nc.gpsimd.dma_start(z_sb[:], z)
    zt_ps = tpp.tile([P, H], F32R, name="ztps", tag="tp")
    nc.tensor.matmul(zt_ps[0:ZD, 0:P], z_sb[:], idr[:], is_transpose=True)
    zT_sb = const.tile([16, P], F32R, name="zT_sb")
    nc.scalar.copy(zT_sb[:], zt_ps[0:ZD, 0:P])
    nc.gpsimd.dma_start(WzT[:, 0:9], Wz.rearrange("a b -> b a"))
    nc.gpsimd.dma_start(WzT[:, 9:16], Wz.rearrange("a b -> b a")[:, 2:9])
    nc.gpsimd.dma_start(bz_sb[:, 0:9], bz)
    nc.gpsimd.dma_start(bz_sb[:, 9:16], bz[:, 2:9])
    zb_ps = tpp.tile([P, H], F32, name="zbps", tag="tp")
    nc.tensor.matmul(zb_ps[:, 0:16], zT_sb[:], WzT[:], start=True, stop=False)
    nc.tensor.matmul(zb_ps[:, 0:16], ones0, bz_sb[:], start=False, stop=True)
    nc.scalar.copy(zb_sb[:, 0:9], zb_ps[:, 0:9])  # rounds to f32r

    # -- x_future + z_bias[:,1:9]; transpose into inT_all rows 0..7 --
    x_sb = ldtmp.tile([P, S * EXO], F32R, name="x_sb", tag="xsb")
    nc.gpsimd.dma_start(x_sb[:], xf)
    x3 = x_sb.rearrange("p (t e) -> p t e", e=EXO)
    zb3 = zb_sb[:, 1:9].unsqueeze(1).broadcast_to((P, S, EXO))
    nc.vector.tensor_tensor(x3, x3, zb3, op=OP.add)
    tpb = P // EXO  # 16 t-values per 128-wide transpose block
    xt_sb = ldtmp.tile([P, max(P, S * EXO)], F32R, name="xt_sb", tag="xsb")
    if S % tpb == 0:
        nxb = (S * EXO) // P  # number of 128-wide transpose blocks (S=64 -> 4)
        for j in range(nxb):
            xt_ps = tpp.tile([P, H], F32R, name="xtps", tag="tp")
            nc.tensor.matmul(
                xt_ps[:, 0:P], x_sb[:, j * P : (j + 1) * P], idr[:],
                is_transpose=True, start=True, stop=True,
            )
            nc.scalar.copy(xt_sb[:, j * P : (j + 1) * P], xt_ps[:, 0:P])
        src4 = xt_sb[:, 0 : S * EXO].rearrange("p (j b) -> p j b", b=P)
        dst4 = inT_x[0:8, :].rearrange("p (j tm b) -> p tm j b", tm=tpb, b=P)
        for tm in range(tpb):
            dma(dst4[:, tm], src4[tm * EXO : (tm + 1) * EXO, :])
    else:
        assert S < tpb
        xt_ps = tpp.tile([P, H], F32R, name="xtps", tag="tp")
        nc.tensor.matmul(
            xt_ps[0 : S * EXO, 0:P], x_sb[:], idr[:],
            is_transpose=True, start=True, stop=True,
        )
        nc.scalar.copy(xt_sb[0 : S * EXO, 0:P], xt_ps[0 : S * EXO, 0:P])
        for tm in range(S):
            dma(
                inT_x[0:8, tm * P : (tm + 1) * P],
                xt_sb[tm * EXO : (tm + 1) * EXO, 0:P],
            )
    # ones row (partition 40): copy from ones0 via DMA
    for s in range(S):
        dma(inT_x[8:9, s * P : (s + 1) * P], ones0)

    # -- y row for t=0: y0.T + zb[:,0].T  (row 0, partition 0) --
    zb0t_ps = tpp.tile([P, H], F32R, name="zb0tps", tag="tp")
    nc.tensor.matmul(zb0t_ps[0:1, 0:P], zb_sb[:, 0:1], idr[:], is_transpose=True)
    zb0row = const.tile([1, P], F32, name="zb0row")
    nc.scalar.copy(zb0row[:], f(zb0t_ps)[0:1, 0:P])
    y0_sb = ldtmp.tile([P, 1], F32R, name="y0sb", tag="y0sb")
    nc.gpsimd.dma_start(y0_sb[:], y0)


    # -- bp_row / ybias_row --
    bp_sb = const.tile([1, 1], F32, name="bp_sb")
    dma(bp_sb[:], bp)
    nc.vector.tensor_copy(bp_row[:], bp_sb[0:1, 0:1].broadcast_to((1, P)))
    nc.vector.tensor_tensor(ybias_row[:], zb0row[:], bp_row[:], op=OP.add)

    # -- ones_row (partition 32): copy from ones0 via DMA --
    dma(ones_row, ones0)

    # -- WpT: WpT[p, k] = Wp[0, k*128+p] --
    nc.gpsimd.dma_start(WpT, Wp.rearrange("o (k p) -> p (o k)", p=P))

    # -- Wih0a10: row 0 = W_ih0[:,0].T (y col), rows 1-8 = W_ih0[:,1:9].T,
    #    row 9 = b_ih0+b_hh0 (columns gate-reordered) --
    for g in range(4):
        cdst = NEWPOS[g] * H
        src = Wih0[g * H : (g + 1) * H, 1:9].rearrange("a b -> b a")
        nc.gpsimd.dma_start(Wih0a9[0:8, cdst : cdst + H], src)
        src0 = Wih0[g * H : (g + 1) * H, 0:1].rearrange("a b -> b a")
        nc.gpsimd.dma_start(wcol_row[:, cdst : cdst + H], src0)

    # bias rows: load both bias vectors into gate-permuted [4, 512] staging
    # tiles (partition g' = NEWPOS[g]), add, then cast-DMA to the f32r row.
    def bias_row(b_a, b_b, dst_row):
        t1 = ldtmp.tile([4, H], F32, name="bs1", tag="ldtmp")
        t2 = ldtmp.tile([4, H], F32, name="bs2", tag="ldtmp")
        for g in range(4):
            dma(t1[NEWPOS[g] : NEWPOS[g] + 1, :], b_a[:, g * H : (g + 1) * H])
            dma(t2[NEWPOS[g] : NEWPOS[g] + 1, :], b_b[:, g * H : (g + 1) * H])
        nc.vector.tensor_tensor(t1[:], t1[:], t2[:], op=OP.add)
        for b in range(4):
            nc.gpsimd.dma_start(
                dst_row[:, b * H : (b + 1) * H], t1[b : b + 1, :]
            )

    bias_row(bih0, bhh0, Wih0a9[8:9, :])
    bias_row(bih1, bhh1, b1row)

    # -- big transposed weights: W [G, H] -> WT[p, k*G + dstblk*128 + c] --
    def build_WT(Wsrc, WT):
        WT4 = WT.rearrange("p (k mb mc) -> p k mb mc", k=KC, mc=P)
        for rr in range(16):
            wt = ldtmp.tile([P, H], F32R, name="wld", tag="ldtmp")
            nc.gpsimd.dma_start(wt[:], Wsrc[rr * P : (rr + 1) * P, :])
            tp_t = tpp.tile([P, H], F32R, name="wtps", tag="tp")
            for k in range(KC):
                nc.tensor.matmul(
                    tp_t[:, k * P : (k + 1) * P], wt[:, k * P : (k + 1) * P], idr[:],
                    is_transpose=True, start=(k == 0), stop=(k == KC - 1),
                )
            dstblk = NEWPOS[rr // 4] * 4 + (rr % 4)
            nc.scalar.copy(
                WT4[:, :, dstblk, :],
                tp_t.rearrange("p (k c) -> p k c", k=KC),
            )

    build_WT(Whh0, WhhT0)
    build_WT(Wih1, WihT1)
    build_WT(Whh1, WhhT1)

    # -- initial states + transposes --
    def load_state(src, tag):
        t = state.tile([P, H], F32R, name=tag, tag=tag)
        nc.gpsimd.dma_start(t[:], src)
        return t

    def transpose_state(h, tag):
        tp_t = tpp.tile([P, H], F32R, name=tag + "ps", tag="tp")
        for k in range(KC):
            nc.tensor.matmul(
                tp_t[:, k * P : (k + 1) * P], h[:, k * P : (k + 1) * P], idr[:],
                is_transpose=True, start=(k == 0), stop=(k == KC - 1),
            )
        hT = state.tile([P, H], F32R, name=tag, tag=tag)
        nc.scalar.copy(hT[:], tp_t[:])
        return hT

    # =========================== STEP LOOP ===========================
    # Gate-tile processing order: tanh tile (g) first so the DVE chain can
    # start while later sigma tiles are still in the matmul stream.
    N_ORDER = [3, 0, 1, 2]

    def lstm_tail(sg, tg, c_prev, ctag, ttag):
        """c' = sg[f]*c + sg[i]*tg ; hT = (sg[o].T) * tanh(c').T directly in
        transposed layout (no untransposed h is ever materialized)."""
        tmp = gact.tile([P, H], F32, name="tmp", tag="tmp")
        nc.vector.tensor_tensor(tmp[:], sg[0][:], tg[:], op=OP.mult)
        c_n = state.tile([P, H], F32R, name=ctag, tag=ctag)
        nc.vector.tensor_tensor(c_n[:], sg[1][:], c_prev[:], op=OP.mult)
        nc.vector.tensor_tensor(c_n[:], c_n[:], tmp[:], op=OP.add)
        # tanh(c') transposes+evicts early (thc lands before sigma_o);
        # sigma_o transposes stay in PSUM and feed the hT multiply directly.
        thc = gact.tile([P, H], F32R, name="thc", tag="thc")
        nc.scalar.activation(thc[:], c_n[:], AF.Tanh)
        th_tp = gsig.tile([P, H], F32R, name="thtp", tag="gsig")
        for k in range(KC):
            s = slice(k * P, (k + 1) * P)
            nc.tensor.matmul(
                th_tp[:, s], thc[:, s], idr[:],
                is_transpose=True, start=(k == 0), stop=(k == KC - 1),
            )
        thT = gact.tile([P, H], F32R, name="thT", tag="soT")
        nc.scalar.copy(thT[:], th_tp[:])
        so_tp = tpp.tile([P, H], F32R, name="sotp", tag="tp")
        for k in range(KC):
            s = slice(k * P, (k + 1) * P)
            nc.tensor.matmul(
                so_tp[:, s], sg[2][:, s], idr[:],
                is_transpose=True, start=(k == 0), stop=(k == KC - 1),
            )
        hT = state.tile([P, H], F32R, name=ttag, tag=ttag)
        nc.vector.tensor_tensor(hT[:], thT[:], so_tp[:], op=OP.mult)
        return c_n, hT

    for rep in range(repeat):
        h0_c = load_state(h00, "h0")
        h1_c = load_state(h01, "h1")
        c0_c = load_state(c00, "c0")
        c1_c = load_state(c01, "c1")
        h0T_c = transpose_state(h0_c, "h0T")
        h1T_c = transpose_state(h1_c, "h1T")
        zz_ps = tpp.tile([P, H], F32R, name="y0tps2", tag="tp")
        nc.tensor.matmul(zz_ps[0:1, 0:P], y0_sb[:], idr[:], is_transpose=True)
        yrow_c = yo.tile([1, P], F32R, name="yrow", tag="yrow")
        nc.vector.tensor_tensor(yrow_c[:], f(zz_ps)[0:1, 0:P], zb0row[:], op=OP.add)

        for t in range(S):
            # ---- layer 0 gates: per-tile [4 hidden MMs, K=10 input MM] + ACT ----
            gs0 = gsig.tile([P, 3 * H], F32, name="gs0", tag="gsig")
            gt0 = gtan.tile([P, H], F32, name="gt0", tag="gtan")
            g0 = lambda n: gs0[:, n * H : (n + 1) * H] if n < 3 else gt0[:]
            lx = inT_x[:, t * P : (t + 1) * P]
            ly = yrow_c[:]
            sg = [None] * 3
            tg = None
            if PER_TILE:
                for n in N_ORDER:
                    nc.tensor.matmul(
                        g0(n), lx, Wih0a9[:, n * H : (n + 1) * H], start=True, stop=False
                    )
                    for k in range(KC):
                        nc.tensor.matmul(
                            g0(n), h0T_c[:, k * P : (k + 1) * P],
                            WhhT0[:, k * G + n * H : k * G + (n + 1) * H],
                            start=False, stop=False,
                        )
                    nc.tensor.matmul(
                        g0(n), ly, wcol_row[0:1, n * H : (n + 1) * H], start=False, stop=True
                    )
                    if n == 3:
                        tg = gact.tile([P, H], F32, name="tg", tag="tg")
                        nc.scalar.activation(tg[:], gt0[:], AF.Tanh)
                    else:
                        dt = F32R if n == 2 else F32
                        sg[n] = gact.tile([P, H], dt, name=f"sg{n}", tag=f"sg{n}")
                        nc.scalar.activation(sg[n][:], gs0[:, n * H : (n + 1) * H], AF.Sigmoid)
            else:
                for n in range(NT):
                    nc.tensor.matmul(
                        g0(n), lx, Wih0a9[:, n * H : (n + 1) * H], start=True, stop=False
                    )
                for k in range(KC):
                    for n in range(NT):
                        nc.tensor.matmul(
                            g0(n), h0T_c[:, k * P : (k + 1) * P],
                            WhhT0[:, k * G + n * H : k * G + (n + 1) * H],
                            start=False, stop=False,
                        )
                for n in range(NT):
                    nc.tensor.matmul(
                        g0(n), ly, wcol_row[0:1, n * H : (n + 1) * H], start=False, stop=True
                    )
                tg = gact.tile([P, H], F32, name="tg", tag="tg")
                nc.scalar.activation(tg[:], gt0[:], AF.Tanh)
                for n in range(3):
                    dt = F32R if n == 2 else F32
                    sg[n] = gact.tile([P, H], dt, name=f"sg{n}", tag=f"sg{n}")
                    nc.scalar.activation(sg[n][:], gs0[:, n * H : (n + 1) * H], AF.Sigmoid)

            c0_c, h0T_n = lstm_tail(sg, tg, c0_c, "c0", "h0T")
            h0T_c = h0T_n

            # ---- layer 1 gates: per-tile [4 h1 MMs, 4 h0 MMs, bias MM] + ACT ----
            gs1 = gsig.tile([P, 3 * H], F32, name="gs1", tag="gsig")
            gt1 = gtan.tile([P, H], F32, name="gt1", tag="gtan")
            g1 = lambda n: gs1[:, n * H : (n + 1) * H] if n < 3 else gt1[:]
            sg1 = [None] * 3
            tg1 = None
            if PER_TILE:
                for n in N_ORDER:
                    for k in range(KC):
                        nc.tensor.matmul(
                            g1(n), h1T_c[:, k * P : (k + 1) * P],
                            WhhT1[:, k * G + n * H : k * G + (n + 1) * H],
                            start=(k == 0), stop=False,
                        )
                    for k in range(KC):
                        nc.tensor.matmul(
                            g1(n), h0T_n[:, k * P : (k + 1) * P],
                            WihT1[:, k * G + n * H : k * G + (n + 1) * H],
                            start=False, stop=False,
                        )
                    nc.tensor.matmul(
                        g1(n), ones_row, b1row[:, n * H : (n + 1) * H], start=False, stop=True
                    )
                    if n == 3:
                        tg1 = gact.tile([P, H], F32, name="tg", tag="tg")
                        nc.scalar.activation(tg1[:], gt1[:], AF.Tanh)
                    else:
                        dt = F32R if n == 2 else F32
                        sg1[n] = gact.tile([P, H], dt, name=f"sg{n}", tag=f"sg{n}")
                        nc.scalar.activation(sg1[n][:], gs1[:, n * H : (n + 1) * H], AF.Sigmoid)
            else:
                for k in range(KC):
                    for n in range(NT):
                        nc.tensor.matmul(
                            g1(n), h1T_c[:, k * P : (k + 1) * P],
                            WhhT1[:, k * G + n * H : k * G + (n + 1) * H],
                            start=(k == 0), stop=False,
                        )
                for k in range(KC):
                    for n in range(NT):
                        nc.tensor.matmul(
                            g1(n), h0T_n[:, k * P : (k + 1) * P],
                            WihT1[:, k * G + n * H : k * G + (n + 1) * H],
                            start=False, stop=False,
                        )
                for n in range(NT):
                    nc.tensor.matmul(
                        g1(n), ones_row, b1row[:, n * H : (n + 1) * H], start=False, stop=True
                    )
                tg1 = gact.tile([P, H], F32, name="tg", tag="tg")
                nc.scalar.activation(tg1[:], gt1[:], AF.Tanh)
                for n in range(3):
                    dt = F32R if n == 2 else F32
                    sg1[n] = gact.tile([P, H], dt, name=f"sg{n}", tag=f"sg{n}")
                    nc.scalar.activation(sg1[n][:], gs1[:, n * H : (n + 1) * H], AF.Sigmoid)

            c1_c, h1T_n = lstm_tail(sg1, tg1, c1_c, "c1", "h1T")
            h1T_c = h1T_n

            # ---- projection: yT = Wp @ h1.T  ([1, 128]) ----
            ytp = tpp.tile([P, H], F32, name="ytp", tag="tp")
            for k in range(KC):
                nc.tensor.matmul(
                    ytp[0:1, 0:P], WpT[:, k : k + 1], h1T_n[:, k * P : (k + 1) * P],
                    start=(k == 0), stop=(k == KC - 1),
                )
            y_pure = yo.tile([1, P], F32, name="yout", tag="yout")
            nc.vector.tensor_tensor(y_pure[:], ytp[0:1, 0:P], bp_row[:], op=OP.add)
            dma(out_d[t : t + 1, :], y_pure[:])
            if t + 1 < S:
                yrow_c = yo.tile([1, P], F32R, name="yrow", tag="yrow")
                nc.vector.tensor_tensor(
                    yrow_c[:], ytp[0:1, 0:P], ybias_row[:], op=OP.add
                )


def make_nc(steps: int, repeat: int = 1):
    nc = bacc.Bacc("TRN2", target_bir_lowering=False, debug=False)
    build_kernel(nc, steps, repeat)
    nc.compile()
    return nc


def shard_inputs(inputs, steps: int):
    """Full inputs dict -> list of 8 per-core input maps."""
    B = inputs["y0"].shape[0]
    nb = B // P
    maps = []
    fa = lambda x: np.ascontiguousarray(np.asarray(x, dtype=np.float32))
    for i in range(nb):
        s = slice(i * P, (i + 1) * P)
        m = {
            "y0": fa(inputs["y0"][s]),
            "x_future": fa(inputs["x_future"][s, :steps].reshape(P, steps * EXO)),
            "h00": fa(inputs["h0"][0, s]),
            "h01": fa(inputs["h0"][1, s]),
            "c00": fa(inputs["c0"][0, s]),
            "c01": fa(inputs["c0"][1, s]),
            "z": fa(inputs["z"][s]),
            "W_ih0": fa(inputs["W_ih0"]),
            "W_hh0": fa(inputs["W_hh0"]),
            "b_ih0": fa(inputs["b_ih0"]).reshape(1, G),
            "b_hh0": fa(inputs["b_hh0"]).reshape(1, G),
            "W_ih1": fa(inputs["W_ih1"]),
            "W_hh1": fa(inputs["W_hh1"]),
            "b_ih1": fa(inputs["b_ih1"]).reshape(1, G),
            "b_hh1": fa(inputs["b_hh1"]).reshape(1, G),
            "W_proj": fa(inputs["W_proj"]),
            "b_proj": fa(inputs["b_proj"]).reshape(1, 1),
            "W_z": fa(inputs["W_z"]),
            "b_z": fa(inputs["b_z"]).reshape(1, 9),
        }
        maps.append(m)
    return maps


def assemble_output(results, steps: int):
    """list of per-core {"out": [P, S]} -> [B, S, 1]."""
    outs = [np.ascontiguousarray(np.asarray(rm["out"]).T).reshape(P, steps, 1) for rm in results]
    return np.concatenate(outs, axis=0)


# ======================= public entry point =======================
_NC_CACHE = {}


def _get_nc():
    if "nc" not in _NC_CACHE:
        _NC_CACHE["nc"] = make_nc(STEPS)
    return _NC_CACHE["nc"]


STEPS = 64
N_CORES = 8


def kernel(**inputs):
    """Full-input entry point: shards batch over 8 NeuronCores, runs the
    Bass LSTM-decoder kernel, reassembles [B, steps, 1] float32 output."""
    from concourse.bass_utils import run_bass_kernel_spmd

    steps = int(inputs.get("steps", STEPS))
    assert steps == STEPS, f"kernel compiled for {STEPS} steps, got {steps}"
    nc = _get_nc()
    maps = shard_inputs(inputs, STEPS)
    res = run_bass_kernel_spmd(nc, maps, list(range(N_CORES)))
    return assemble_output(res.results, STEPS).astype(np.float32)

